# revision 14
# baseline (speedup 1.0000x reference)
"""BernNet GNN message-passing kernel for 8 Trainium2 NeuronCores.

Math: reference computes out = sum_m C(K,m)/2^K * relu(temp)[m] * L^m M^{K-m} x
with L = I - Ahat, M = I + Ahat (Ahat = D^-1/2 A D^-1/2) and x = MLP(node_feat).
L and M commute, so out = p(Ahat) x for a degree-K polynomial p whose monomial
coefficients c_j are an exact (host-side, fp64) linear function of relu(temp).
That needs K=10 sparse aggregations instead of the reference's 65.

v2 vs baseline:
- z gather table in fp16 (128B rows): halves gather + AllGather traffic.
- dsq factored out of the output accumulation (out = dsq * sum_j c_j*st_j +
  c0*x), so per-iteration scaling is a few fused wide DVE ops per destination
  half instead of 3 narrow ops per chunk.
- Gathers batched over chunk ranges (one indirect DMA per ~5 chunks) to cut
  gpsimd dispatch overhead.
- AllGather split by destination half into two collectives writing disjoint
  row slices of one z_full tensor; the first (large) one is issued as soon as
  the low chunks are done and overlaps the tail of the chunk loop.
"""

import math

import numpy as np

import concourse.bass as bass
import concourse.mybir as mybir
import concourse.tile as tile
from concourse import bacc
from concourse import bass_utils

# Problem constants (hardcoded per contract; kernel.py must be self-contained)
N = 100000
E = 3200000
K = 10
D_IN = 512
D_H = 256
F = 64

NC = 8          # cores
P = 128         # partitions
NPC_REAL = N // NC          # 12500 real nodes per core
NCHUNK = (NPC_REAL + P - 1) // P   # 98
NPC = NCHUNK * P            # 12544 padded nodes per core
SHARD = NPC + 1             # +1 zero row (for padding slots)
ZROWS = NC * SHARD
ZPAD = NPC                  # index of core 0's zero row (used for all pads)
KSPLIT = 64                 # chunks [0,KSPLIT) ship in the first AllGather
HALF = KSPLIT * P

F32 = mybir.dt.float32
F16 = mybir.dt.float16
I32 = mybir.dt.int32

BATCH = 5                   # chunks per indirect-gather DMA


def _poly_coeffs(temp: np.ndarray) -> np.ndarray:
    """Monomial coefficients c_j of p(t) = sum_m C(K,m)/2^K relu(temp)[m] (1-t)^m (1+t)^(K-m)."""
    T = np.maximum(temp.astype(np.float64), 0.0)
    c = np.zeros(K + 1, dtype=np.float64)
    for m in range(K + 1):
        a = np.array([1.0])
        for _ in range(m):
            a = np.convolve(a, [1.0, -1.0])   # * (1 - t)
        for _ in range(K - m):
            a = np.convolve(a, [1.0, 1.0])    # * (1 + t)
        c += (math.comb(K, m) / float(2 ** K)) * T[m] * a
    return c


def _host_prep(node_feat, edge_index, temp):
    """Permutation, CSR slot structure, and per-core input shards."""
    row = np.asarray(edge_index[0], dtype=np.int64)
    col = np.asarray(edge_index[1], dtype=np.int64)
    deg = np.bincount(row, minlength=N).astype(np.int64)

    # pi: node -> global padded position. Core c owns originals [c*12500,(c+1)*12500),
    # sorted ascending by degree within the core; pads sit at the low ranks.
    pos = np.empty(N, dtype=np.int64)
    npad = NPC - NPC_REAL
    for c in range(NC):
        ids = np.arange(c * NPC_REAL, (c + 1) * NPC_REAL)
        order = np.argsort(deg[ids], kind="stable")
        pos[ids[order]] = c * NPC + npad + np.arange(NPC_REAL)

    pd = pos[row]
    ps = pos[col]
    order = np.argsort(pd, kind="stable")
    pd_s = pd[order]
    ps_s = ps[order]
    cnt = np.bincount(pd_s, minlength=NC * NPC).astype(np.int64)
    rowptr = np.concatenate([[0], np.cumsum(cnt)])
    slot = np.arange(E, dtype=np.int64) - rowptr[pd_s]

    c_e = pd_s // NPC
    r_e = pd_s % NPC
    k_e = r_e // P
    p_e = r_e % P

    # shared-across-cores slot counts per chunk
    S_arr = np.zeros((NC, NCHUNK), dtype=np.int64)
    np.maximum.at(S_arr, (c_e, k_e), slot + 1)
    S_k = np.maximum(S_arr.max(axis=0), 1).astype(np.int64)
    off = np.concatenate([[0], np.cumsum(S_k)])
    total_S = int(off[-1])

    # table row of pi-position (c, r) is c*SHARD + r (shards carry a zero row)
    ps_row = (ps_s // NPC) * SHARD + (ps_s % NPC)
    idx_all = np.full((NC, P, total_S), ZPAD, dtype=np.int32)
    idx_all[c_e, p_e, off[k_e] + slot] = ps_row.astype(np.int32)

    degpk = cnt.reshape(NC, NCHUNK, P).transpose(0, 2, 1).astype(np.float32)
    degpk = np.ascontiguousarray(degpk)

    nfT = np.zeros((NC, D_IN, NPC), dtype=np.float32)
    cc = pos // NPC
    rr = pos % NPC
    nfT[cc, :, rr] = np.asarray(node_feat, dtype=np.float32)

    cj = _poly_coeffs(np.asarray(temp))
    return dict(
        pos=pos, S_k=S_k, off=off, total_S=total_S,
        idx_all=idx_all, degpk=degpk, nfT=nfT, cj=cj,
    )


def _build_nc(S_k, off, total_S, cj):
    """Build the Bass module (shared across all 8 cores)."""
    nc = bacc.Bacc("TRN2", target_bir_lowering=False, debug=False, num_devices=NC)

    nfT_d = nc.dram_tensor("nfT", [D_IN, NPC], F32, kind="ExternalInput")
    idx_d = nc.dram_tensor("idx", [P, total_S], I32, kind="ExternalInput")
    deg_d = nc.dram_tensor("degpk", [P, NCHUNK], F32, kind="ExternalInput")
    W1_d = nc.dram_tensor("W1", [D_IN, D_H], F32, kind="ExternalInput")
    b1_d = nc.dram_tensor("b1", [D_H], F32, kind="ExternalInput")
    W2_d = nc.dram_tensor("W2", [D_H, F], F32, kind="ExternalInput")
    b2_d = nc.dram_tensor("b2", [F], F32, kind="ExternalInput")
    out_d = nc.dram_tensor("out", [NPC, F], F32, kind="ExternalOutput")

    from concourse.masks import make_identity

    def batches(k0, k1):
        return [(b, min(b + BATCH, k1)) for b in range(k0, k1, BATCH)]

    G_max = max(int(off[b1] - off[b0]) for b0, b1 in batches(0, NCHUNK))

    with tile.TileContext(nc) as tc:
        with (
            tc.tile_pool(name="consts", bufs=1) as consts,
            tc.tile_pool(name="dram", bufs=1, space="DRAM") as dram,
            tc.tile_pool(name="psum", bufs=2, space="PSUM") as psum,
            tc.tile_pool(name="gp", bufs=2) as gp,
            tc.tile_pool(name="sp", bufs=2) as sp,
        ):
            # one Shared AllGather output per iteration; two collectives per
            # iteration write disjoint row slices (single writer per slice)
            z_fulls = [
                dram.tile([ZROWS, F], F16, addr_space="Shared", name=f"z_full_{j}")
                for j in range(K)
            ]
            z_shard = dram.tile([SHARD, F], F16, name="z_shard")

            # ---- resident constants ----
            idx_sb = consts.tile([P, total_S], I32, name="idx_sb")
            nc.sync.dma_start(out=idx_sb[:], in_=idx_d[:])
            deg_sb = consts.tile([P, NCHUNK], F32, name="deg_sb")
            nc.sync.dma_start(out=deg_sb[:], in_=deg_d[:])

            mask = consts.tile([P, NCHUNK], F32, name="mask")
            nc.vector.tensor_scalar(out=mask[:], in0=deg_sb[:], scalar1=0.0,
                                    scalar2=None, op0=mybir.AluOpType.is_gt)
            dsq = consts.tile([P, NCHUNK], F32, name="dsq")
            nc.vector.tensor_scalar_max(out=dsq[:], in0=deg_sb[:], scalar1=1.0)
            nc.scalar.activation(out=dsq[:], in_=dsq[:],
                                 func=mybir.ActivationFunctionType.Sqrt)
            nc.vector.reciprocal(out=dsq[:], in_=dsq[:])
            nc.vector.tensor_tensor(out=dsq[:], in0=dsq[:], in1=mask[:],
                                    op=mybir.AluOpType.mult)
            dinv = consts.tile([P, NCHUNK], F32, name="dinv")
            nc.vector.tensor_tensor(out=dinv[:], in0=dsq[:], in1=dsq[:],
                                    op=mybir.AluOpType.mult)

            # F-replicated 1/deg (fp16) for the wide per-half zt multiply
            dinv_f = consts.tile([P, NCHUNK * F], F16, name="dinv_f")
            for k in range(NCHUNK):
                nc.vector.tensor_copy(
                    out=dinv_f[:, k * F:(k + 1) * F],
                    in_=dinv[:, k:k + 1].to_broadcast([P, F]))

            # out_acc accumulates sum_j c_j * st_j; x_all holds c0 * x
            out_acc = consts.tile([P, NCHUNK * F], F32, name="out_acc")
            nc.vector.memset(out_acc[:], 0.0)
            x_all = consts.tile([P, NCHUNK * F], F32, name="x_all")
            st_all = consts.tile([P, NCHUNK * F], F32, name="st_all")

            # zero row of this core's shard (gathered by padding slots)
            ztile = consts.tile([1, F], F16, name="ztile")
            nc.vector.memset(ztile[:], 0.0)
            nc.sync.dma_start(out=z_shard[NPC:NPC + 1, :], in_=ztile[:])

            ident = consts.tile([P, P], F32, name="ident")
            make_identity(nc, ident[:])

            c0 = float(cj[0])
            rg = [list(range(NC))]

            # ---- MLP: x^T = W2^T relu(W1^T nfT + b1) + b2, then per-128 transpose ----
            with (
                tc.tile_pool(name="mlp", bufs=3) as mlp,
                tc.tile_pool(name="mlpc", bufs=1) as mlpc,
            ):
                w1 = []  # w1[h][k]: [128(K), 128(M=channels h*128..)]
                for h in range(D_H // P):
                    w1.append([])
                    for k in range(D_IN // P):
                        t = mlpc.tile([P, P], F32, name=f"w1_{h}_{k}")
                        nc.sync.dma_start(
                            out=t[:], in_=W1_d[k * P:(k + 1) * P, h * P:(h + 1) * P])
                        w1[h].append(t)
                w2 = []
                for k in range(D_H // P):
                    t = mlpc.tile([P, F], F32, name=f"w2_{k}")
                    nc.sync.dma_start(out=t[:], in_=W2_d[k * P:(k + 1) * P, :])
                    w2.append(t)
                # biases as flat rows; applied as a K=1 matmul against a ones-row
                b1r = []
                for h in range(D_H // P):
                    t = mlpc.tile([1, P], F32, name=f"b1r_{h}")
                    nc.sync.dma_start(out=t[:], in_=b1_d[None, h * P:(h + 1) * P])
                    b1r.append(t)
                b2r = mlpc.tile([1, F], F32, name="b2r")
                nc.sync.dma_start(out=b2r[:], in_=b2_d[None, :])
                ones = mlpc.tile([1, 512], F32, name="ones")
                nc.vector.memset(ones[:], 1.0)

                ntiles = []
                nleft = NPC
                while nleft > 0:
                    t = min(512, nleft)
                    ntiles.append(t)
                    nleft -= t
                n0 = 0
                for NT in ntiles:
                    nf = []
                    for k in range(D_IN // P):
                        t = mlp.tile([P, 512], F32, tag="nf", name=f"nf_{n0}_{k}")
                        nc.sync.dma_start(
                            out=t[:, :NT], in_=nfT_d[k * P:(k + 1) * P, n0:n0 + NT])
                        nf.append(t)
                    hs = []
                    for h in range(D_H // P):
                        hp = psum.tile([P, 512], F32, tag="hpsum", name=f"hp_{n0}_{h}")
                        for k in range(D_IN // P):
                            nc.tensor.matmul(
                                out=hp[:, :NT], lhsT=w1[h][k][:], rhs=nf[k][:, :NT],
                                start=(k == 0), stop=False)
                        nc.tensor.matmul(
                            out=hp[:, :NT], lhsT=b1r[h][:], rhs=ones[:, :NT],
                            start=False, stop=True)
                        ht = mlp.tile([P, 512], F32, tag=f"h{h}", name=f"h_{n0}_{h}")
                        nc.scalar.activation(
                            out=ht[:, :NT], in_=hp[:, :NT],
                            func=mybir.ActivationFunctionType.Relu,
                            bias=0.0, scale=1.0)
                        hs.append(ht)
                    xp = psum.tile([F, 512], F32, tag="xpsum", name=f"xp_{n0}")
                    for k in range(D_H // P):
                        nc.tensor.matmul(
                            out=xp[:, :NT], lhsT=w2[k][:], rhs=hs[k][:, :NT],
                            start=(k == 0), stop=False)
                    nc.tensor.matmul(
                        out=xp[:, :NT], lhsT=b2r[:], rhs=ones[:, :NT],
                        start=False, stop=True)
                    xt = mlp.tile([F, 512], F32, tag="xt", name=f"xt_{n0}")
                    nc.scalar.activation(
                        out=xt[:, :NT], in_=xp[:, :NT],
                        func=mybir.ActivationFunctionType.Copy,
                        bias=0.0, scale=1.0)
                    for b in range(NT // P):
                        kc = n0 // P + b
                        tp = psum.tile([P, F], F32, tag="tp", name=f"tp_{kc}")
                        nc.tensor.transpose(
                            out=tp[:], in_=xt[:, b * P:(b + 1) * P],
                            identity=ident[:F, :F])
                        nc.vector.tensor_scalar_mul(
                            out=x_all[:, kc * F:(kc + 1) * F], in0=tp[:], scalar1=c0)
                        z0 = sp.tile([P, F], F16, tag="z0", name=f"z0_{kc}")
                        nc.vector.tensor_scalar(
                            out=z0[:], in0=tp[:], scalar1=dsq[:, kc:kc + 1],
                            scalar2=None, op0=mybir.AluOpType.mult)
                        nc.sync.dma_start(
                            out=z_shard[kc * P:(kc + 1) * P, :], in_=z0[:])
                    n0 += NT
                nc.gpsimd.collective_compute(
                    "AllGather", mybir.AluOpType.bypass, replica_groups=rg,
                    ins=[z_shard[:].opt()], outs=[z_fulls[0][:].opt()])

            # ---- K aggregation iterations ----
            for j in range(1, K + 1):
                z_src = z_fulls[j - 1]
                cjf = float(cj[j])

                for b0, b1 in batches(0, NCHUNK):
                    o0 = int(off[b0])
                    o1 = int(off[b1])
                    g = gp.tile([P, G_max * F], F16, tag="g",
                                name=f"g_{j}_{b0}")
                    nc.gpsimd.indirect_dma_start(
                        out=g[:, :(o1 - o0) * F], out_offset=None,
                        in_=z_src[:],
                        in_offset=bass.IndirectOffsetOnAxis(
                            ap=idx_sb[:, o0:o1], axis=0),
                    )
                    for k in range(b0, b1):
                        Sk = int(S_k[k])
                        o = int(off[k]) - o0
                        nc.vector.tensor_reduce(
                            out=st_all[:, k * F:(k + 1) * F],
                            in_=g[:, o * F:(o + Sk) * F].rearrange(
                                "p (s f) -> p f s", f=F),
                            axis=mybir.AxisListType.X, op=mybir.AluOpType.add)
                # out_acc += c_j * st (fused)
                nc.vector.scalar_tensor_tensor(
                    out=out_acc[:], in0=st_all[:], scalar=cjf,
                    in1=out_acc[:],
                    op0=mybir.AluOpType.mult, op1=mybir.AluOpType.add)
                if j < K:
                    # z_j = dinv * st (fp16) -> shard -> AllGather
                    zt = sp.tile([P, NCHUNK * F], F16, tag="zt",
                                 name=f"zt_{j}")
                    nc.vector.tensor_tensor(
                        out=zt[:], in0=st_all[:],
                        in1=dinv_f[:], op=mybir.AluOpType.mult)
                    nc.sync.dma_start(
                        out=z_shard[0:NPC, :].rearrange(
                            "(k p) f -> p k f", p=P),
                        in_=zt[:].rearrange("p (k f) -> p k f", f=F))
                    nc.gpsimd.collective_compute(
                        "AllGather", mybir.AluOpType.bypass,
                        replica_groups=rg,
                        ins=[z_shard[:].opt()],
                        outs=[z_fulls[j][:].opt()])

            # ---- finalize: out = dsq * out_acc + c0*x, store ----
            for k in range(NCHUNK):
                nc.vector.scalar_tensor_tensor(
                    out=x_all[:, k * F:(k + 1) * F],
                    in0=out_acc[:, k * F:(k + 1) * F],
                    scalar=dsq[:, k:k + 1],
                    in1=x_all[:, k * F:(k + 1) * F],
                    op0=mybir.AluOpType.mult, op1=mybir.AluOpType.add)
            nc.sync.dma_start(
                out=out_d[:].rearrange("(k p) f -> p k f", p=P),
                in_=x_all[:].rearrange("p (k f) -> p k f", f=F))

    nc.compile()
    return nc


def _build_mlp_nc(c0):
    """MLP-only module for the degenerate polynomial case (p(t) == c0):
    out = c0 * (relu(nfT^T W1 + b1) W2 + b2). No graph aggregation at all.
    Matmuls in fp16 (PSUM accumulates fp32); ~0.1% error vs the 2e-2 budget.
    """
    nc = bacc.Bacc("TRN2", target_bir_lowering=False, debug=False, num_devices=NC)

    nfT_d = nc.dram_tensor("nfT", [D_IN, NPC], F16, kind="ExternalInput")
    W1_d = nc.dram_tensor("W1", [D_IN, D_H], F16, kind="ExternalInput")
    b1_d = nc.dram_tensor("b1", [D_H], F16, kind="ExternalInput")
    W2_d = nc.dram_tensor("W2", [D_H, F], F16, kind="ExternalInput")
    b2_d = nc.dram_tensor("b2", [F], F16, kind="ExternalInput")
    out_d = nc.dram_tensor("out", [NPC, F], F32, kind="ExternalOutput")

    from concourse.masks import make_identity

    with tile.TileContext(nc) as tc:
        with (
            tc.tile_pool(name="consts", bufs=1) as consts,
            tc.tile_pool(name="psum", bufs=2, space="PSUM") as psum,
            tc.tile_pool(name="mlp", bufs=3) as mlp,
        ):
            ident = consts.tile([P, P], F32, name="ident")
            make_identity(nc, ident[:])
            x_all = consts.tile([P, NCHUNK * F], F32, name="x_all")

            w1 = []
            for h in range(D_H // P):
                w1.append([])
                for k in range(D_IN // P):
                    t = consts.tile([P, P], F16, name=f"w1_{h}_{k}")
                    nc.sync.dma_start(
                        out=t[:], in_=W1_d[k * P:(k + 1) * P, h * P:(h + 1) * P])
                    w1[h].append(t)
            w2 = []
            for k in range(D_H // P):
                t = consts.tile([P, F], F16, name=f"w2_{k}")
                nc.sync.dma_start(out=t[:], in_=W2_d[k * P:(k + 1) * P, :])
                w2.append(t)
            b1r = []
            for h in range(D_H // P):
                t = consts.tile([1, P], F16, name=f"b1r_{h}")
                nc.sync.dma_start(out=t[:], in_=b1_d[None, h * P:(h + 1) * P])
                b1r.append(t)
            b2r = consts.tile([1, F], F16, name="b2r")
            nc.sync.dma_start(out=b2r[:], in_=b2_d[None, :])
            ones = consts.tile([1, 512], F16, name="ones")
            nc.vector.memset(ones[:], 1.0)

            n0 = 0
            while n0 < NPC:
                NT = min(512, NPC - n0)
                nf = []
                for k in range(D_IN // P):
                    t = mlp.tile([P, 512], F16, tag="nf", name=f"nf_{n0}_{k}")
                    nc.sync.dma_start(
                        out=t[:, :NT], in_=nfT_d[k * P:(k + 1) * P, n0:n0 + NT])
                    nf.append(t)
                hs = []
                for h in range(D_H // P):
                    hp = psum.tile([P, 512], F32, tag="hpsum", name=f"hp_{n0}_{h}")
                    for k in range(D_IN // P):
                        nc.tensor.matmul(
                            out=hp[:, :NT], lhsT=w1[h][k][:], rhs=nf[k][:, :NT],
                            start=(k == 0), stop=False)
                    nc.tensor.matmul(
                        out=hp[:, :NT], lhsT=b1r[h][:], rhs=ones[:, :NT],
                        start=False, stop=True)
                    ht = mlp.tile([P, 512], F16, tag=f"h{h}", name=f"h_{n0}_{h}")
                    nc.scalar.activation(
                        out=ht[:, :NT], in_=hp[:, :NT],
                        func=mybir.ActivationFunctionType.Relu,
                        bias=0.0, scale=1.0)
                    hs.append(ht)
                xp = psum.tile([F, 512], F32, tag="xpsum", name=f"xp_{n0}")
                for k in range(D_H // P):
                    nc.tensor.matmul(
                        out=xp[:, :NT], lhsT=w2[k][:], rhs=hs[k][:, :NT],
                        start=(k == 0), stop=False)
                nc.tensor.matmul(
                    out=xp[:, :NT], lhsT=b2r[:], rhs=ones[:, :NT],
                    start=False, stop=True)
                xt = mlp.tile([F, 512], F32, tag="xt", name=f"xt_{n0}")
                nc.scalar.activation(
                    out=xt[:, :NT], in_=xp[:, :NT],
                    func=mybir.ActivationFunctionType.Copy,
                    bias=0.0, scale=1.0)
                for b in range(NT // P):
                    kc = n0 // P + b
                    tp = psum.tile([P, F], F32, tag="tp", name=f"tp_{kc}")
                    nc.tensor.transpose(
                        out=tp[:], in_=xt[:, b * P:(b + 1) * P],
                        identity=ident[:F, :F])
                    nc.vector.tensor_scalar_mul(
                        out=x_all[:, kc * F:(kc + 1) * F], in0=tp[:], scalar1=c0)
                n0 += NT
            nc.sync.dma_start(
                out=out_d[:].rearrange("(k p) f -> p k f", p=P),
                in_=x_all[:].rearrange("p (k f) -> p k f", f=F))

    nc.compile()
    return nc


_CACHE = {}


def kernel(node_feat, edge_index, W1, b1, W2, b2, temp):
    node_feat = np.asarray(node_feat, dtype=np.float32)
    edge_index = np.asarray(edge_index)
    W1 = np.ascontiguousarray(np.asarray(W1, dtype=np.float32))
    b1 = np.ascontiguousarray(np.asarray(b1, dtype=np.float32))
    W2 = np.ascontiguousarray(np.asarray(W2, dtype=np.float32))
    b2 = np.ascontiguousarray(np.asarray(b2, dtype=np.float32))
    temp = np.asarray(temp, dtype=np.float32)

    cj = _poly_coeffs(temp)
    degenerate = bool(np.max(np.abs(cj[1:])) <= 1e-9 * max(abs(cj[0]), 1.0))

    global LAST_RESULTS
    if degenerate:
        # p(t) == c0 identically: the aggregation contributes exactly
        # c_j * (...) = 0 for every j >= 1, so out = c0 * MLP(node_feat).
        nfT = np.zeros((NC, D_IN, NPC), dtype=np.float16)
        nf = node_feat.T.astype(np.float16)  # [D_IN, N]
        for c in range(NC):
            nfT[c, :, :NPC_REAL] = nf[:, c * NPC_REAL:(c + 1) * NPC_REAL]
        key = ("mlp", float(cj[0]))
        nc = _CACHE.get(key)
        if nc is None:
            nc = _build_mlp_nc(float(cj[0]))
            _CACHE[key] = nc
        W1h = W1.astype(np.float16)
        b1h = b1.astype(np.float16)
        W2h = W2.astype(np.float16)
        b2h = b2.astype(np.float16)
        in_maps = []
        for c in range(NC):
            in_maps.append({
                "nfT": np.ascontiguousarray(nfT[c]),
                "W1": W1h, "b1": b1h, "W2": W2h, "b2": b2h,
            })
        res = bass_utils.run_bass_kernel_spmd(nc, in_maps,
                                              core_ids=list(range(NC)))
        LAST_RESULTS = res
        out_cat = np.concatenate(
            [r["out"][:NPC_REAL] for r in res.results], axis=0)
        return np.ascontiguousarray(out_cat)

    prep = _host_prep(node_feat, edge_index, temp)

    key = (edge_index.tobytes()[:4096], temp.tobytes())
    nc = _CACHE.get(key)
    if nc is None:
        nc = _build_nc(prep["S_k"], prep["off"], prep["total_S"], prep["cj"])
        _CACHE[key] = nc

    in_maps = []
    for c in range(NC):
        in_maps.append({
            "nfT": np.ascontiguousarray(prep["nfT"][c]),
            "idx": np.ascontiguousarray(prep["idx_all"][c]),
            "degpk": np.ascontiguousarray(prep["degpk"][c]),
            "W1": W1, "b1": b1, "W2": W2, "b2": b2,
        })

    res = bass_utils.run_bass_kernel_spmd(nc, in_maps, core_ids=list(range(NC)))
    LAST_RESULTS = res
    out_cat = np.concatenate([r["out"] for r in res.results], axis=0)
    return np.ascontiguousarray(out_cat[prep["pos"]])


LAST_RESULTS = None


# revision 17
# speedup vs baseline: 1.2160x; 1.2160x over previous
"""BernNet GNN message-passing kernel for 8 Trainium2 NeuronCores.

Math: reference computes out = sum_m C(K,m)/2^K * relu(temp)[m] * L^m M^{K-m} x
with L = I - Ahat, M = I + Ahat (Ahat = D^-1/2 A D^-1/2) and x = MLP(node_feat).
L and M commute, so out = p(Ahat) x for a degree-K polynomial p whose monomial
coefficients c_j are an exact (host-side, fp64) linear function of relu(temp).
That needs K=10 sparse aggregations instead of the reference's 65.

v2 vs baseline:
- z gather table in fp16 (128B rows): halves gather + AllGather traffic.
- dsq factored out of the output accumulation (out = dsq * sum_j c_j*st_j +
  c0*x), so per-iteration scaling is a few fused wide DVE ops per destination
  half instead of 3 narrow ops per chunk.
- Gathers batched over chunk ranges (one indirect DMA per ~5 chunks) to cut
  gpsimd dispatch overhead.
- AllGather split by destination half into two collectives writing disjoint
  row slices of one z_full tensor; the first (large) one is issued as soon as
  the low chunks are done and overlaps the tail of the chunk loop.
"""

import math

import numpy as np

import concourse.bass as bass
import concourse.mybir as mybir
import concourse.tile as tile
from concourse import bacc
from concourse import bass_utils

# Problem constants (hardcoded per contract; kernel.py must be self-contained)
N = 100000
E = 3200000
K = 10
D_IN = 512
D_H = 256
F = 64

NC = 8          # cores
P = 128         # partitions
NPC_REAL = N // NC          # 12500 real nodes per core
NCHUNK = (NPC_REAL + P - 1) // P   # 98
NPC = NCHUNK * P            # 12544 padded nodes per core
SHARD = NPC + 1             # +1 zero row (for padding slots)
ZROWS = NC * SHARD
ZPAD = NPC                  # index of core 0's zero row (used for all pads)
KSPLIT = 64                 # chunks [0,KSPLIT) ship in the first AllGather
HALF = KSPLIT * P

F32 = mybir.dt.float32
F16 = mybir.dt.float16
I32 = mybir.dt.int32

BATCH = 5                   # chunks per indirect-gather DMA


def _poly_coeffs(temp: np.ndarray) -> np.ndarray:
    """Monomial coefficients c_j of p(t) = sum_m C(K,m)/2^K relu(temp)[m] (1-t)^m (1+t)^(K-m)."""
    T = np.maximum(temp.astype(np.float64), 0.0)
    c = np.zeros(K + 1, dtype=np.float64)
    for m in range(K + 1):
        a = np.array([1.0])
        for _ in range(m):
            a = np.convolve(a, [1.0, -1.0])   # * (1 - t)
        for _ in range(K - m):
            a = np.convolve(a, [1.0, 1.0])    # * (1 + t)
        c += (math.comb(K, m) / float(2 ** K)) * T[m] * a
    return c


def _host_prep(node_feat, edge_index, temp):
    """Permutation, CSR slot structure, and per-core input shards."""
    row = np.asarray(edge_index[0], dtype=np.int64)
    col = np.asarray(edge_index[1], dtype=np.int64)
    deg = np.bincount(row, minlength=N).astype(np.int64)

    # pi: node -> global padded position. Core c owns originals [c*12500,(c+1)*12500),
    # sorted ascending by degree within the core; pads sit at the low ranks.
    pos = np.empty(N, dtype=np.int64)
    npad = NPC - NPC_REAL
    for c in range(NC):
        ids = np.arange(c * NPC_REAL, (c + 1) * NPC_REAL)
        order = np.argsort(deg[ids], kind="stable")
        pos[ids[order]] = c * NPC + npad + np.arange(NPC_REAL)

    pd = pos[row]
    ps = pos[col]
    order = np.argsort(pd, kind="stable")
    pd_s = pd[order]
    ps_s = ps[order]
    cnt = np.bincount(pd_s, minlength=NC * NPC).astype(np.int64)
    rowptr = np.concatenate([[0], np.cumsum(cnt)])
    slot = np.arange(E, dtype=np.int64) - rowptr[pd_s]

    c_e = pd_s // NPC
    r_e = pd_s % NPC
    k_e = r_e // P
    p_e = r_e % P

    # shared-across-cores slot counts per chunk
    S_arr = np.zeros((NC, NCHUNK), dtype=np.int64)
    np.maximum.at(S_arr, (c_e, k_e), slot + 1)
    S_k = np.maximum(S_arr.max(axis=0), 1).astype(np.int64)
    off = np.concatenate([[0], np.cumsum(S_k)])
    total_S = int(off[-1])

    # table row of pi-position (c, r) is c*SHARD + r (shards carry a zero row)
    ps_row = (ps_s // NPC) * SHARD + (ps_s % NPC)
    idx_all = np.full((NC, P, total_S), ZPAD, dtype=np.int32)
    idx_all[c_e, p_e, off[k_e] + slot] = ps_row.astype(np.int32)

    degpk = cnt.reshape(NC, NCHUNK, P).transpose(0, 2, 1).astype(np.float32)
    degpk = np.ascontiguousarray(degpk)

    nfT = np.zeros((NC, D_IN, NPC), dtype=np.float32)
    cc = pos // NPC
    rr = pos % NPC
    nfT[cc, :, rr] = np.asarray(node_feat, dtype=np.float32)

    cj = _poly_coeffs(np.asarray(temp))
    return dict(
        pos=pos, S_k=S_k, off=off, total_S=total_S,
        idx_all=idx_all, degpk=degpk, nfT=nfT, cj=cj,
    )


def _build_nc(S_k, off, total_S, cj):
    """Build the Bass module (shared across all 8 cores)."""
    nc = bacc.Bacc("TRN2", target_bir_lowering=False, debug=False, num_devices=NC)

    nfT_d = nc.dram_tensor("nfT", [D_IN, NPC], F32, kind="ExternalInput")
    idx_d = nc.dram_tensor("idx", [P, total_S], I32, kind="ExternalInput")
    deg_d = nc.dram_tensor("degpk", [P, NCHUNK], F32, kind="ExternalInput")
    W1_d = nc.dram_tensor("W1", [D_IN, D_H], F32, kind="ExternalInput")
    b1_d = nc.dram_tensor("b1", [D_H], F32, kind="ExternalInput")
    W2_d = nc.dram_tensor("W2", [D_H, F], F32, kind="ExternalInput")
    b2_d = nc.dram_tensor("b2", [F], F32, kind="ExternalInput")
    out_d = nc.dram_tensor("out", [NPC, F], F32, kind="ExternalOutput")

    from concourse.masks import make_identity

    def batches(k0, k1):
        return [(b, min(b + BATCH, k1)) for b in range(k0, k1, BATCH)]

    G_max = max(int(off[b1] - off[b0]) for b0, b1 in batches(0, NCHUNK))

    with tile.TileContext(nc) as tc:
        with (
            tc.tile_pool(name="consts", bufs=1) as consts,
            tc.tile_pool(name="dram", bufs=1, space="DRAM") as dram,
            tc.tile_pool(name="psum", bufs=2, space="PSUM") as psum,
            tc.tile_pool(name="gp", bufs=2) as gp,
            tc.tile_pool(name="sp", bufs=2) as sp,
        ):
            # one Shared AllGather output per iteration; two collectives per
            # iteration write disjoint row slices (single writer per slice)
            z_fulls = [
                dram.tile([ZROWS, F], F16, addr_space="Shared", name=f"z_full_{j}")
                for j in range(K)
            ]
            z_shard = dram.tile([SHARD, F], F16, name="z_shard")

            # ---- resident constants ----
            idx_sb = consts.tile([P, total_S], I32, name="idx_sb")
            nc.sync.dma_start(out=idx_sb[:], in_=idx_d[:])
            deg_sb = consts.tile([P, NCHUNK], F32, name="deg_sb")
            nc.sync.dma_start(out=deg_sb[:], in_=deg_d[:])

            mask = consts.tile([P, NCHUNK], F32, name="mask")
            nc.vector.tensor_scalar(out=mask[:], in0=deg_sb[:], scalar1=0.0,
                                    scalar2=None, op0=mybir.AluOpType.is_gt)
            dsq = consts.tile([P, NCHUNK], F32, name="dsq")
            nc.vector.tensor_scalar_max(out=dsq[:], in0=deg_sb[:], scalar1=1.0)
            nc.scalar.activation(out=dsq[:], in_=dsq[:],
                                 func=mybir.ActivationFunctionType.Sqrt)
            nc.vector.reciprocal(out=dsq[:], in_=dsq[:])
            nc.vector.tensor_tensor(out=dsq[:], in0=dsq[:], in1=mask[:],
                                    op=mybir.AluOpType.mult)
            dinv = consts.tile([P, NCHUNK], F32, name="dinv")
            nc.vector.tensor_tensor(out=dinv[:], in0=dsq[:], in1=dsq[:],
                                    op=mybir.AluOpType.mult)

            # F-replicated 1/deg (fp16) for the wide per-half zt multiply
            dinv_f = consts.tile([P, NCHUNK * F], F16, name="dinv_f")
            for k in range(NCHUNK):
                nc.vector.tensor_copy(
                    out=dinv_f[:, k * F:(k + 1) * F],
                    in_=dinv[:, k:k + 1].to_broadcast([P, F]))

            # out_acc accumulates sum_j c_j * st_j; x_all holds c0 * x
            out_acc = consts.tile([P, NCHUNK * F], F32, name="out_acc")
            nc.vector.memset(out_acc[:], 0.0)
            x_all = consts.tile([P, NCHUNK * F], F32, name="x_all")
            st_all = consts.tile([P, NCHUNK * F], F32, name="st_all")

            # zero row of this core's shard (gathered by padding slots)
            ztile = consts.tile([1, F], F16, name="ztile")
            nc.vector.memset(ztile[:], 0.0)
            nc.sync.dma_start(out=z_shard[NPC:NPC + 1, :], in_=ztile[:])

            ident = consts.tile([P, P], F32, name="ident")
            make_identity(nc, ident[:])

            c0 = float(cj[0])
            rg = [list(range(NC))]

            # ---- MLP: x^T = W2^T relu(W1^T nfT + b1) + b2, then per-128 transpose ----
            with (
                tc.tile_pool(name="mlp", bufs=3) as mlp,
                tc.tile_pool(name="mlpc", bufs=1) as mlpc,
            ):
                w1 = []  # w1[h][k]: [128(K), 128(M=channels h*128..)]
                for h in range(D_H // P):
                    w1.append([])
                    for k in range(D_IN // P):
                        t = mlpc.tile([P, P], F32, name=f"w1_{h}_{k}")
                        nc.sync.dma_start(
                            out=t[:], in_=W1_d[k * P:(k + 1) * P, h * P:(h + 1) * P])
                        w1[h].append(t)
                w2 = []
                for k in range(D_H // P):
                    t = mlpc.tile([P, F], F32, name=f"w2_{k}")
                    nc.sync.dma_start(out=t[:], in_=W2_d[k * P:(k + 1) * P, :])
                    w2.append(t)
                # biases as flat rows; applied as a K=1 matmul against a ones-row
                b1r = []
                for h in range(D_H // P):
                    t = mlpc.tile([1, P], F32, name=f"b1r_{h}")
                    nc.sync.dma_start(out=t[:], in_=b1_d[None, h * P:(h + 1) * P])
                    b1r.append(t)
                b2r = mlpc.tile([1, F], F32, name="b2r")
                nc.sync.dma_start(out=b2r[:], in_=b2_d[None, :])
                ones = mlpc.tile([1, 512], F32, name="ones")
                nc.vector.memset(ones[:], 1.0)

                ntiles = []
                nleft = NPC
                while nleft > 0:
                    t = min(512, nleft)
                    ntiles.append(t)
                    nleft -= t
                n0 = 0
                for NT in ntiles:
                    nf = []
                    for k in range(D_IN // P):
                        t = mlp.tile([P, 512], F32, tag="nf", name=f"nf_{n0}_{k}")
                        nc.sync.dma_start(
                            out=t[:, :NT], in_=nfT_d[k * P:(k + 1) * P, n0:n0 + NT])
                        nf.append(t)
                    hs = []
                    for h in range(D_H // P):
                        hp = psum.tile([P, 512], F32, tag="hpsum", name=f"hp_{n0}_{h}")
                        for k in range(D_IN // P):
                            nc.tensor.matmul(
                                out=hp[:, :NT], lhsT=w1[h][k][:], rhs=nf[k][:, :NT],
                                start=(k == 0), stop=False)
                        nc.tensor.matmul(
                            out=hp[:, :NT], lhsT=b1r[h][:], rhs=ones[:, :NT],
                            start=False, stop=True)
                        ht = mlp.tile([P, 512], F32, tag=f"h{h}", name=f"h_{n0}_{h}")
                        nc.scalar.activation(
                            out=ht[:, :NT], in_=hp[:, :NT],
                            func=mybir.ActivationFunctionType.Relu,
                            bias=0.0, scale=1.0)
                        hs.append(ht)
                    xp = psum.tile([F, 512], F32, tag="xpsum", name=f"xp_{n0}")
                    for k in range(D_H // P):
                        nc.tensor.matmul(
                            out=xp[:, :NT], lhsT=w2[k][:], rhs=hs[k][:, :NT],
                            start=(k == 0), stop=False)
                    nc.tensor.matmul(
                        out=xp[:, :NT], lhsT=b2r[:], rhs=ones[:, :NT],
                        start=False, stop=True)
                    xt = mlp.tile([F, 512], F32, tag="xt", name=f"xt_{n0}")
                    nc.scalar.activation(
                        out=xt[:, :NT], in_=xp[:, :NT],
                        func=mybir.ActivationFunctionType.Copy,
                        bias=0.0, scale=1.0)
                    for b in range(NT // P):
                        kc = n0 // P + b
                        tp = psum.tile([P, F], F32, tag="tp", name=f"tp_{kc}")
                        nc.tensor.transpose(
                            out=tp[:], in_=xt[:, b * P:(b + 1) * P],
                            identity=ident[:F, :F])
                        nc.vector.tensor_scalar_mul(
                            out=x_all[:, kc * F:(kc + 1) * F], in0=tp[:], scalar1=c0)
                        z0 = sp.tile([P, F], F16, tag="z0", name=f"z0_{kc}")
                        nc.vector.tensor_scalar(
                            out=z0[:], in0=tp[:], scalar1=dsq[:, kc:kc + 1],
                            scalar2=None, op0=mybir.AluOpType.mult)
                        nc.sync.dma_start(
                            out=z_shard[kc * P:(kc + 1) * P, :], in_=z0[:])
                    n0 += NT
                nc.gpsimd.collective_compute(
                    "AllGather", mybir.AluOpType.bypass, replica_groups=rg,
                    ins=[z_shard[:].opt()], outs=[z_fulls[0][:].opt()])

            # ---- K aggregation iterations ----
            for j in range(1, K + 1):
                z_src = z_fulls[j - 1]
                cjf = float(cj[j])

                for b0, b1 in batches(0, NCHUNK):
                    o0 = int(off[b0])
                    o1 = int(off[b1])
                    g = gp.tile([P, G_max * F], F16, tag="g",
                                name=f"g_{j}_{b0}")
                    nc.gpsimd.indirect_dma_start(
                        out=g[:, :(o1 - o0) * F], out_offset=None,
                        in_=z_src[:],
                        in_offset=bass.IndirectOffsetOnAxis(
                            ap=idx_sb[:, o0:o1], axis=0),
                    )
                    for k in range(b0, b1):
                        Sk = int(S_k[k])
                        o = int(off[k]) - o0
                        nc.vector.tensor_reduce(
                            out=st_all[:, k * F:(k + 1) * F],
                            in_=g[:, o * F:(o + Sk) * F].rearrange(
                                "p (s f) -> p f s", f=F),
                            axis=mybir.AxisListType.X, op=mybir.AluOpType.add)
                # out_acc += c_j * st (fused)
                nc.vector.scalar_tensor_tensor(
                    out=out_acc[:], in0=st_all[:], scalar=cjf,
                    in1=out_acc[:],
                    op0=mybir.AluOpType.mult, op1=mybir.AluOpType.add)
                if j < K:
                    # z_j = dinv * st (fp16) -> shard -> AllGather
                    zt = sp.tile([P, NCHUNK * F], F16, tag="zt",
                                 name=f"zt_{j}")
                    nc.vector.tensor_tensor(
                        out=zt[:], in0=st_all[:],
                        in1=dinv_f[:], op=mybir.AluOpType.mult)
                    nc.sync.dma_start(
                        out=z_shard[0:NPC, :].rearrange(
                            "(k p) f -> p k f", p=P),
                        in_=zt[:].rearrange("p (k f) -> p k f", f=F))
                    nc.gpsimd.collective_compute(
                        "AllGather", mybir.AluOpType.bypass,
                        replica_groups=rg,
                        ins=[z_shard[:].opt()],
                        outs=[z_fulls[j][:].opt()])

            # ---- finalize: out = dsq * out_acc + c0*x, store ----
            for k in range(NCHUNK):
                nc.vector.scalar_tensor_tensor(
                    out=x_all[:, k * F:(k + 1) * F],
                    in0=out_acc[:, k * F:(k + 1) * F],
                    scalar=dsq[:, k:k + 1],
                    in1=x_all[:, k * F:(k + 1) * F],
                    op0=mybir.AluOpType.mult, op1=mybir.AluOpType.add)
            nc.sync.dma_start(
                out=out_d[:].rearrange("(k p) f -> p k f", p=P),
                in_=x_all[:].rearrange("p (k f) -> p k f", f=F))

    nc.compile()
    return nc


def _build_mlp_nc(c0, has_bias):
    """MLP-only module for the degenerate polynomial case (p(t) == c0):
    out^T = c0 * (W2^T relu(W1^T nfT + b1) + b2). No graph aggregation.
    Matmuls in fp16 (PSUM accumulates fp32); ~0.1% error vs the 2e-2 budget.
    The output stays feature-major [F, NPC]; the host transposes.
    """
    nc = bacc.Bacc("TRN2", target_bir_lowering=False, debug=False, num_devices=NC)

    nfT_d = nc.dram_tensor("nfT", [D_IN, NPC], F16, kind="ExternalInput")
    W1_d = nc.dram_tensor("W1", [D_IN, D_H], F16, kind="ExternalInput")
    b1_d = nc.dram_tensor("b1", [D_H], F16, kind="ExternalInput")
    W2_d = nc.dram_tensor("W2", [D_H, F], F16, kind="ExternalInput")
    b2_d = nc.dram_tensor("b2", [F], F16, kind="ExternalInput")
    outT_d = nc.dram_tensor("outT", [F, NPC], F32, kind="ExternalOutput")

    with tile.TileContext(nc) as tc:
        with (
            tc.tile_pool(name="consts", bufs=1) as consts,
            tc.tile_pool(name="psum", bufs=2, space="PSUM") as psum,
            tc.tile_pool(name="mlp", bufs=3) as mlp,
        ):
            w1 = []
            for h in range(D_H // P):
                w1.append([])
                for k in range(D_IN // P):
                    t = consts.tile([P, P], F16, name=f"w1_{h}_{k}")
                    nc.sync.dma_start(
                        out=t[:], in_=W1_d[k * P:(k + 1) * P, h * P:(h + 1) * P])
                    w1[h].append(t)
            w2 = []
            for k in range(D_H // P):
                t = consts.tile([P, F], F16, name=f"w2_{k}")
                nc.sync.dma_start(out=t[:], in_=W2_d[k * P:(k + 1) * P, :])
                w2.append(t)
            if has_bias:
                b1r = []
                for h in range(D_H // P):
                    t = consts.tile([1, P], F16, name=f"b1r_{h}")
                    nc.sync.dma_start(out=t[:], in_=b1_d[None, h * P:(h + 1) * P])
                    b1r.append(t)
                b2r = consts.tile([1, F], F16, name="b2r")
                nc.sync.dma_start(out=b2r[:], in_=b2_d[None, :])
                ones = consts.tile([1, 512], F16, name="ones")
                nc.vector.memset(ones[:], 1.0)

            n0 = 0
            while n0 < NPC:
                NT = min(512, NPC - n0)
                nf = []
                for k in range(D_IN // P):
                    t = mlp.tile([P, 512], F16, tag="nf", name=f"nf_{n0}_{k}")
                    nc.sync.dma_start(
                        out=t[:, :NT], in_=nfT_d[k * P:(k + 1) * P, n0:n0 + NT])
                    nf.append(t)
                hs = []
                for h in range(D_H // P):
                    hp = psum.tile([P, 512], F32, tag="hpsum", name=f"hp_{n0}_{h}")
                    nk = D_IN // P
                    for k in range(nk):
                        nc.tensor.matmul(
                            out=hp[:, :NT], lhsT=w1[h][k][:], rhs=nf[k][:, :NT],
                            start=(k == 0),
                            stop=(not has_bias and k == nk - 1))
                    if has_bias:
                        nc.tensor.matmul(
                            out=hp[:, :NT], lhsT=b1r[h][:], rhs=ones[:, :NT],
                            start=False, stop=True)
                    ht = mlp.tile([P, 512], F16, tag=f"h{h}", name=f"h_{n0}_{h}")
                    nc.scalar.activation(
                        out=ht[:, :NT], in_=hp[:, :NT],
                        func=mybir.ActivationFunctionType.Relu,
                        bias=0.0, scale=1.0)
                    hs.append(ht)
                xp = psum.tile([F, 512], F32, tag="xpsum", name=f"xp_{n0}")
                nk = D_H // P
                for k in range(nk):
                    nc.tensor.matmul(
                        out=xp[:, :NT], lhsT=w2[k][:], rhs=hs[k][:, :NT],
                        start=(k == 0),
                        stop=(not has_bias and k == nk - 1))
                if has_bias:
                    nc.tensor.matmul(
                        out=xp[:, :NT], lhsT=b2r[:], rhs=ones[:, :NT],
                        start=False, stop=True)
                xt = mlp.tile([F, 512], F32, tag="xt", name=f"xt_{n0}")
                nc.scalar.activation(
                    out=xt[:, :NT], in_=xp[:, :NT],
                    func=mybir.ActivationFunctionType.Copy,
                    bias=0.0, scale=c0)
                nc.sync.dma_start(
                    out=outT_d[:, n0:n0 + NT], in_=xt[:, :NT])
                n0 += NT

    nc.compile()
    return nc


_CACHE = {}


def kernel(node_feat, edge_index, W1, b1, W2, b2, temp):
    node_feat = np.asarray(node_feat, dtype=np.float32)
    edge_index = np.asarray(edge_index)
    W1 = np.ascontiguousarray(np.asarray(W1, dtype=np.float32))
    b1 = np.ascontiguousarray(np.asarray(b1, dtype=np.float32))
    W2 = np.ascontiguousarray(np.asarray(W2, dtype=np.float32))
    b2 = np.ascontiguousarray(np.asarray(b2, dtype=np.float32))
    temp = np.asarray(temp, dtype=np.float32)

    cj = _poly_coeffs(temp)
    degenerate = bool(np.max(np.abs(cj[1:])) <= 1e-9 * max(abs(cj[0]), 1.0))
    import os as _os
    if _os.environ.get("KFORCE_GENERAL", "") == "1":
        degenerate = False

    global LAST_RESULTS
    if degenerate:
        # p(t) == c0 identically: the aggregation contributes exactly
        # c_j * (...) = 0 for every j >= 1, so out = c0 * MLP(node_feat).
        nfT = np.zeros((NC, D_IN, NPC), dtype=np.float16)
        nf = node_feat.T.astype(np.float16)  # [D_IN, N]
        for c in range(NC):
            nfT[c, :, :NPC_REAL] = nf[:, c * NPC_REAL:(c + 1) * NPC_REAL]
        has_bias = bool(np.any(b1) or np.any(b2))
        key = ("mlp", float(cj[0]), has_bias)
        nc = _CACHE.get(key)
        if nc is None:
            nc = _build_mlp_nc(float(cj[0]), has_bias)
            _CACHE[key] = nc
        W1h = W1.astype(np.float16)
        b1h = b1.astype(np.float16)
        W2h = W2.astype(np.float16)
        b2h = b2.astype(np.float16)
        in_maps = []
        for c in range(NC):
            in_maps.append({
                "nfT": np.ascontiguousarray(nfT[c]),
                "W1": W1h, "b1": b1h, "W2": W2h, "b2": b2h,
            })
        res = bass_utils.run_bass_kernel_spmd(nc, in_maps,
                                              core_ids=list(range(NC)))
        LAST_RESULTS = res
        out = np.empty((N, F), dtype=np.float32)
        for c in range(NC):
            out[c * NPC_REAL:(c + 1) * NPC_REAL] = \
                np.asarray(res.results[c]["outT"])[:, :NPC_REAL].T
        return out

    prep = _host_prep(node_feat, edge_index, temp)

    key = (edge_index.tobytes()[:4096], temp.tobytes())
    nc = _CACHE.get(key)
    if nc is None:
        nc = _build_nc(prep["S_k"], prep["off"], prep["total_S"], prep["cj"])
        _CACHE[key] = nc

    in_maps = []
    for c in range(NC):
        in_maps.append({
            "nfT": np.ascontiguousarray(prep["nfT"][c]),
            "idx": np.ascontiguousarray(prep["idx_all"][c]),
            "degpk": np.ascontiguousarray(prep["degpk"][c]),
            "W1": W1, "b1": b1, "W2": W2, "b2": b2,
        })

    res = bass_utils.run_bass_kernel_spmd(nc, in_maps, core_ids=list(range(NC)))
    LAST_RESULTS = res
    out_cat = np.concatenate([r["out"] for r in res.results], axis=0)
    return np.ascontiguousarray(out_cat[prep["pos"]])


LAST_RESULTS = None


# revision 19
# speedup vs baseline: 2.1147x; 1.7390x over previous
"""BernNet GNN message-passing kernel for 8 Trainium2 NeuronCores.

Math: reference computes out = sum_m C(K,m)/2^K * relu(temp)[m] * L^m M^{K-m} x
with L = I - Ahat, M = I + Ahat (Ahat = D^-1/2 A D^-1/2) and x = MLP(node_feat).
L and M commute, so out = p(Ahat) x for a degree-K polynomial p whose monomial
coefficients c_j are an exact (host-side, fp64) linear function of relu(temp).
That needs K=10 sparse aggregations instead of the reference's 65.

v2 vs baseline:
- z gather table in fp16 (128B rows): halves gather + AllGather traffic.
- dsq factored out of the output accumulation (out = dsq * sum_j c_j*st_j +
  c0*x), so per-iteration scaling is a few fused wide DVE ops per destination
  half instead of 3 narrow ops per chunk.
- Gathers batched over chunk ranges (one indirect DMA per ~5 chunks) to cut
  gpsimd dispatch overhead.
- AllGather split by destination half into two collectives writing disjoint
  row slices of one z_full tensor; the first (large) one is issued as soon as
  the low chunks are done and overlaps the tail of the chunk loop.
"""

import math

import numpy as np

import concourse.bass as bass
import concourse.mybir as mybir
import concourse.tile as tile
from concourse import bacc
from concourse import bass_utils

# Problem constants (hardcoded per contract; kernel.py must be self-contained)
N = 100000
E = 3200000
K = 10
D_IN = 512
D_H = 256
F = 64

NC = 8          # cores
P = 128         # partitions
NPC_REAL = N // NC          # 12500 real nodes per core
NCHUNK = (NPC_REAL + P - 1) // P   # 98
NPC = NCHUNK * P            # 12544 padded nodes per core
SHARD = NPC + 1             # +1 zero row (for padding slots)
ZROWS = NC * SHARD
ZPAD = NPC                  # index of core 0's zero row (used for all pads)
KSPLIT = 64                 # chunks [0,KSPLIT) ship in the first AllGather
HALF = KSPLIT * P

F32 = mybir.dt.float32
F16 = mybir.dt.float16
I32 = mybir.dt.int32

BATCH = 5                   # chunks per indirect-gather DMA


def _poly_coeffs(temp: np.ndarray) -> np.ndarray:
    """Monomial coefficients c_j of p(t) = sum_m C(K,m)/2^K relu(temp)[m] (1-t)^m (1+t)^(K-m)."""
    T = np.maximum(temp.astype(np.float64), 0.0)
    c = np.zeros(K + 1, dtype=np.float64)
    for m in range(K + 1):
        a = np.array([1.0])
        for _ in range(m):
            a = np.convolve(a, [1.0, -1.0])   # * (1 - t)
        for _ in range(K - m):
            a = np.convolve(a, [1.0, 1.0])    # * (1 + t)
        c += (math.comb(K, m) / float(2 ** K)) * T[m] * a
    return c


def _host_prep(node_feat, edge_index, temp):
    """Permutation, CSR slot structure, and per-core input shards."""
    row = np.asarray(edge_index[0], dtype=np.int64)
    col = np.asarray(edge_index[1], dtype=np.int64)
    deg = np.bincount(row, minlength=N).astype(np.int64)

    # pi: node -> global padded position. Core c owns originals [c*12500,(c+1)*12500),
    # sorted ascending by degree within the core; pads sit at the low ranks.
    pos = np.empty(N, dtype=np.int64)
    npad = NPC - NPC_REAL
    for c in range(NC):
        ids = np.arange(c * NPC_REAL, (c + 1) * NPC_REAL)
        order = np.argsort(deg[ids], kind="stable")
        pos[ids[order]] = c * NPC + npad + np.arange(NPC_REAL)

    pd = pos[row]
    ps = pos[col]
    order = np.argsort(pd, kind="stable")
    pd_s = pd[order]
    ps_s = ps[order]
    cnt = np.bincount(pd_s, minlength=NC * NPC).astype(np.int64)
    rowptr = np.concatenate([[0], np.cumsum(cnt)])
    slot = np.arange(E, dtype=np.int64) - rowptr[pd_s]

    c_e = pd_s // NPC
    r_e = pd_s % NPC
    k_e = r_e // P
    p_e = r_e % P

    # shared-across-cores slot counts per chunk
    S_arr = np.zeros((NC, NCHUNK), dtype=np.int64)
    np.maximum.at(S_arr, (c_e, k_e), slot + 1)
    S_k = np.maximum(S_arr.max(axis=0), 1).astype(np.int64)
    off = np.concatenate([[0], np.cumsum(S_k)])
    total_S = int(off[-1])

    # table row of pi-position (c, r) is c*SHARD + r (shards carry a zero row)
    ps_row = (ps_s // NPC) * SHARD + (ps_s % NPC)
    idx_all = np.full((NC, P, total_S), ZPAD, dtype=np.int32)
    idx_all[c_e, p_e, off[k_e] + slot] = ps_row.astype(np.int32)

    degpk = cnt.reshape(NC, NCHUNK, P).transpose(0, 2, 1).astype(np.float32)
    degpk = np.ascontiguousarray(degpk)

    nfT = np.zeros((NC, D_IN, NPC), dtype=np.float32)
    cc = pos // NPC
    rr = pos % NPC
    nfT[cc, :, rr] = np.asarray(node_feat, dtype=np.float32)

    cj = _poly_coeffs(np.asarray(temp))
    return dict(
        pos=pos, S_k=S_k, off=off, total_S=total_S,
        idx_all=idx_all, degpk=degpk, nfT=nfT, cj=cj,
    )


def _build_nc(S_k, off, total_S, cj):
    """Build the Bass module (shared across all 8 cores)."""
    nc = bacc.Bacc("TRN2", target_bir_lowering=False, debug=False, num_devices=NC)

    nfT_d = nc.dram_tensor("nfT", [D_IN, NPC], F32, kind="ExternalInput")
    idx_d = nc.dram_tensor("idx", [P, total_S], I32, kind="ExternalInput")
    deg_d = nc.dram_tensor("degpk", [P, NCHUNK], F32, kind="ExternalInput")
    W1_d = nc.dram_tensor("W1", [D_IN, D_H], F32, kind="ExternalInput")
    b1_d = nc.dram_tensor("b1", [D_H], F32, kind="ExternalInput")
    W2_d = nc.dram_tensor("W2", [D_H, F], F32, kind="ExternalInput")
    b2_d = nc.dram_tensor("b2", [F], F32, kind="ExternalInput")
    out_d = nc.dram_tensor("out", [NPC, F], F32, kind="ExternalOutput")

    from concourse.masks import make_identity

    def batches(k0, k1):
        return [(b, min(b + BATCH, k1)) for b in range(k0, k1, BATCH)]

    G_max = max(int(off[b1] - off[b0]) for b0, b1 in batches(0, NCHUNK))

    with tile.TileContext(nc) as tc:
        with (
            tc.tile_pool(name="consts", bufs=1) as consts,
            tc.tile_pool(name="dram", bufs=1, space="DRAM") as dram,
            tc.tile_pool(name="psum", bufs=2, space="PSUM") as psum,
            tc.tile_pool(name="gp", bufs=2) as gp,
            tc.tile_pool(name="sp", bufs=2) as sp,
        ):
            # one Shared AllGather output per iteration; two collectives per
            # iteration write disjoint row slices (single writer per slice)
            z_fulls = [
                dram.tile([ZROWS, F], F16, addr_space="Shared", name=f"z_full_{j}")
                for j in range(K)
            ]
            z_shard = dram.tile([SHARD, F], F16, name="z_shard")

            # ---- resident constants ----
            idx_sb = consts.tile([P, total_S], I32, name="idx_sb")
            nc.sync.dma_start(out=idx_sb[:], in_=idx_d[:])
            deg_sb = consts.tile([P, NCHUNK], F32, name="deg_sb")
            nc.sync.dma_start(out=deg_sb[:], in_=deg_d[:])

            mask = consts.tile([P, NCHUNK], F32, name="mask")
            nc.vector.tensor_scalar(out=mask[:], in0=deg_sb[:], scalar1=0.0,
                                    scalar2=None, op0=mybir.AluOpType.is_gt)
            dsq = consts.tile([P, NCHUNK], F32, name="dsq")
            nc.vector.tensor_scalar_max(out=dsq[:], in0=deg_sb[:], scalar1=1.0)
            nc.scalar.activation(out=dsq[:], in_=dsq[:],
                                 func=mybir.ActivationFunctionType.Sqrt)
            nc.vector.reciprocal(out=dsq[:], in_=dsq[:])
            nc.vector.tensor_tensor(out=dsq[:], in0=dsq[:], in1=mask[:],
                                    op=mybir.AluOpType.mult)
            dinv = consts.tile([P, NCHUNK], F32, name="dinv")
            nc.vector.tensor_tensor(out=dinv[:], in0=dsq[:], in1=dsq[:],
                                    op=mybir.AluOpType.mult)

            # F-replicated 1/deg (fp16) for the wide per-half zt multiply
            dinv_f = consts.tile([P, NCHUNK * F], F16, name="dinv_f")
            for k in range(NCHUNK):
                nc.vector.tensor_copy(
                    out=dinv_f[:, k * F:(k + 1) * F],
                    in_=dinv[:, k:k + 1].to_broadcast([P, F]))

            # out_acc accumulates sum_j c_j * st_j; x_all holds c0 * x
            out_acc = consts.tile([P, NCHUNK * F], F32, name="out_acc")
            nc.vector.memset(out_acc[:], 0.0)
            x_all = consts.tile([P, NCHUNK * F], F32, name="x_all")
            st_all = consts.tile([P, NCHUNK * F], F32, name="st_all")

            # zero row of this core's shard (gathered by padding slots)
            ztile = consts.tile([1, F], F16, name="ztile")
            nc.vector.memset(ztile[:], 0.0)
            nc.sync.dma_start(out=z_shard[NPC:NPC + 1, :], in_=ztile[:])

            ident = consts.tile([P, P], F32, name="ident")
            make_identity(nc, ident[:])

            c0 = float(cj[0])
            rg = [list(range(NC))]

            # ---- MLP: x^T = W2^T relu(W1^T nfT + b1) + b2, then per-128 transpose ----
            with (
                tc.tile_pool(name="mlp", bufs=3) as mlp,
                tc.tile_pool(name="mlpc", bufs=1) as mlpc,
            ):
                w1 = []  # w1[h][k]: [128(K), 128(M=channels h*128..)]
                for h in range(D_H // P):
                    w1.append([])
                    for k in range(D_IN // P):
                        t = mlpc.tile([P, P], F32, name=f"w1_{h}_{k}")
                        nc.sync.dma_start(
                            out=t[:], in_=W1_d[k * P:(k + 1) * P, h * P:(h + 1) * P])
                        w1[h].append(t)
                w2 = []
                for k in range(D_H // P):
                    t = mlpc.tile([P, F], F32, name=f"w2_{k}")
                    nc.sync.dma_start(out=t[:], in_=W2_d[k * P:(k + 1) * P, :])
                    w2.append(t)
                # biases as flat rows; applied as a K=1 matmul against a ones-row
                b1r = []
                for h in range(D_H // P):
                    t = mlpc.tile([1, P], F32, name=f"b1r_{h}")
                    nc.sync.dma_start(out=t[:], in_=b1_d[None, h * P:(h + 1) * P])
                    b1r.append(t)
                b2r = mlpc.tile([1, F], F32, name="b2r")
                nc.sync.dma_start(out=b2r[:], in_=b2_d[None, :])
                ones = mlpc.tile([1, 512], F32, name="ones")
                nc.vector.memset(ones[:], 1.0)

                ntiles = []
                nleft = NPC
                while nleft > 0:
                    t = min(512, nleft)
                    ntiles.append(t)
                    nleft -= t
                n0 = 0
                for NT in ntiles:
                    nf = []
                    for k in range(D_IN // P):
                        t = mlp.tile([P, 512], F32, tag="nf", name=f"nf_{n0}_{k}")
                        nc.sync.dma_start(
                            out=t[:, :NT], in_=nfT_d[k * P:(k + 1) * P, n0:n0 + NT])
                        nf.append(t)
                    hs = []
                    for h in range(D_H // P):
                        hp = psum.tile([P, 512], F32, tag="hpsum", name=f"hp_{n0}_{h}")
                        for k in range(D_IN // P):
                            nc.tensor.matmul(
                                out=hp[:, :NT], lhsT=w1[h][k][:], rhs=nf[k][:, :NT],
                                start=(k == 0), stop=False)
                        nc.tensor.matmul(
                            out=hp[:, :NT], lhsT=b1r[h][:], rhs=ones[:, :NT],
                            start=False, stop=True)
                        ht = mlp.tile([P, 512], F32, tag=f"h{h}", name=f"h_{n0}_{h}")
                        nc.scalar.activation(
                            out=ht[:, :NT], in_=hp[:, :NT],
                            func=mybir.ActivationFunctionType.Relu,
                            bias=0.0, scale=1.0)
                        hs.append(ht)
                    xp = psum.tile([F, 512], F32, tag="xpsum", name=f"xp_{n0}")
                    for k in range(D_H // P):
                        nc.tensor.matmul(
                            out=xp[:, :NT], lhsT=w2[k][:], rhs=hs[k][:, :NT],
                            start=(k == 0), stop=False)
                    nc.tensor.matmul(
                        out=xp[:, :NT], lhsT=b2r[:], rhs=ones[:, :NT],
                        start=False, stop=True)
                    xt = mlp.tile([F, 512], F32, tag="xt", name=f"xt_{n0}")
                    nc.scalar.activation(
                        out=xt[:, :NT], in_=xp[:, :NT],
                        func=mybir.ActivationFunctionType.Copy,
                        bias=0.0, scale=1.0)
                    for b in range(NT // P):
                        kc = n0 // P + b
                        tp = psum.tile([P, F], F32, tag="tp", name=f"tp_{kc}")
                        nc.tensor.transpose(
                            out=tp[:], in_=xt[:, b * P:(b + 1) * P],
                            identity=ident[:F, :F])
                        nc.vector.tensor_scalar_mul(
                            out=x_all[:, kc * F:(kc + 1) * F], in0=tp[:], scalar1=c0)
                        z0 = sp.tile([P, F], F16, tag="z0", name=f"z0_{kc}")
                        nc.vector.tensor_scalar(
                            out=z0[:], in0=tp[:], scalar1=dsq[:, kc:kc + 1],
                            scalar2=None, op0=mybir.AluOpType.mult)
                        nc.sync.dma_start(
                            out=z_shard[kc * P:(kc + 1) * P, :], in_=z0[:])
                    n0 += NT
                nc.gpsimd.collective_compute(
                    "AllGather", mybir.AluOpType.bypass, replica_groups=rg,
                    ins=[z_shard[:].opt()], outs=[z_fulls[0][:].opt()])

            # ---- K aggregation iterations ----
            for j in range(1, K + 1):
                z_src = z_fulls[j - 1]
                cjf = float(cj[j])

                for b0, b1 in batches(0, NCHUNK):
                    o0 = int(off[b0])
                    o1 = int(off[b1])
                    g = gp.tile([P, G_max * F], F16, tag="g",
                                name=f"g_{j}_{b0}")
                    nc.gpsimd.indirect_dma_start(
                        out=g[:, :(o1 - o0) * F], out_offset=None,
                        in_=z_src[:],
                        in_offset=bass.IndirectOffsetOnAxis(
                            ap=idx_sb[:, o0:o1], axis=0),
                    )
                    for k in range(b0, b1):
                        Sk = int(S_k[k])
                        o = int(off[k]) - o0
                        nc.vector.tensor_reduce(
                            out=st_all[:, k * F:(k + 1) * F],
                            in_=g[:, o * F:(o + Sk) * F].rearrange(
                                "p (s f) -> p f s", f=F),
                            axis=mybir.AxisListType.X, op=mybir.AluOpType.add)
                # out_acc += c_j * st (fused)
                nc.vector.scalar_tensor_tensor(
                    out=out_acc[:], in0=st_all[:], scalar=cjf,
                    in1=out_acc[:],
                    op0=mybir.AluOpType.mult, op1=mybir.AluOpType.add)
                if j < K:
                    # z_j = dinv * st (fp16) -> shard -> AllGather
                    zt = sp.tile([P, NCHUNK * F], F16, tag="zt",
                                 name=f"zt_{j}")
                    nc.vector.tensor_tensor(
                        out=zt[:], in0=st_all[:],
                        in1=dinv_f[:], op=mybir.AluOpType.mult)
                    nc.sync.dma_start(
                        out=z_shard[0:NPC, :].rearrange(
                            "(k p) f -> p k f", p=P),
                        in_=zt[:].rearrange("p (k f) -> p k f", f=F))
                    nc.gpsimd.collective_compute(
                        "AllGather", mybir.AluOpType.bypass,
                        replica_groups=rg,
                        ins=[z_shard[:].opt()],
                        outs=[z_fulls[j][:].opt()])

            # ---- finalize: out = dsq * out_acc + c0*x, store ----
            for k in range(NCHUNK):
                nc.vector.scalar_tensor_tensor(
                    out=x_all[:, k * F:(k + 1) * F],
                    in0=out_acc[:, k * F:(k + 1) * F],
                    scalar=dsq[:, k:k + 1],
                    in1=x_all[:, k * F:(k + 1) * F],
                    op0=mybir.AluOpType.mult, op1=mybir.AluOpType.add)
            nc.sync.dma_start(
                out=out_d[:].rearrange("(k p) f -> p k f", p=P),
                in_=x_all[:].rearrange("p (k f) -> p k f", f=F))

    nc.compile()
    return nc


def _build_mlp_nc(c0, has_bias):
    """MLP-only module for the degenerate polynomial case (p(t) == c0):
    out^T = c0 * (W2^T relu(W1^T nfT + b1) + b2). No graph aggregation.
    Matmuls in fp16 (PSUM accumulates fp32); ~0.1% error vs the 2e-2 budget.
    The output stays feature-major [F, NPC]; the host transposes.
    """
    nc = bacc.Bacc("TRN2", target_bir_lowering=False, debug=False, num_devices=NC)

    nfT_d = nc.dram_tensor("nfT", [D_IN, NPC], F16, kind="ExternalInput")
    W1_d = nc.dram_tensor("W1", [D_IN, D_H], F16, kind="ExternalInput")
    b1_d = nc.dram_tensor("b1", [D_H], F16, kind="ExternalInput")
    W2_d = nc.dram_tensor("W2", [D_H, F], F16, kind="ExternalInput")
    b2_d = nc.dram_tensor("b2", [F], F16, kind="ExternalInput")
    outT_d = nc.dram_tensor("outT", [F, NPC], F32, kind="ExternalOutput")

    with tile.TileContext(nc) as tc:
        with (
            tc.tile_pool(name="consts", bufs=1) as consts,
            tc.tile_pool(name="psum", bufs=3, space="PSUM") as psum,
            tc.tile_pool(name="mlp", bufs=4) as mlp,
        ):
            w1 = []
            for h in range(D_H // P):
                w1.append([])
                for k in range(D_IN // P):
                    t = consts.tile([P, P], F16, name=f"w1_{h}_{k}")
                    nc.sync.dma_start(
                        out=t[:], in_=W1_d[k * P:(k + 1) * P, h * P:(h + 1) * P])
                    w1[h].append(t)
            w2 = []
            for k in range(D_H // P):
                t = consts.tile([P, F], F16, name=f"w2_{k}")
                nc.sync.dma_start(out=t[:], in_=W2_d[k * P:(k + 1) * P, :])
                w2.append(t)
            if has_bias:
                b1r = []
                for h in range(D_H // P):
                    t = consts.tile([1, P], F16, name=f"b1r_{h}")
                    nc.sync.dma_start(out=t[:], in_=b1_d[None, h * P:(h + 1) * P])
                    b1r.append(t)
                b2r = consts.tile([1, F], F16, name="b2r")
                nc.sync.dma_start(out=b2r[:], in_=b2_d[None, :])
                ones = consts.tile([1, 512], F16, name="ones")
                nc.vector.memset(ones[:], 1.0)

            n0 = 0
            while n0 < NPC:
                NT = min(512, NPC - n0)
                nf = []
                for k in range(D_IN // P):
                    t = mlp.tile([P, 512], F16, tag="nf", name=f"nf_{n0}_{k}")
                    nc.sync.dma_start(
                        out=t[:, :NT], in_=nfT_d[k * P:(k + 1) * P, n0:n0 + NT])
                    nf.append(t)
                hs = []
                for h in range(D_H // P):
                    hp = psum.tile([P, 512], F32, tag="hpsum", name=f"hp_{n0}_{h}")
                    nk = D_IN // P
                    for k in range(nk):
                        nc.tensor.matmul(
                            out=hp[:, :NT], lhsT=w1[h][k][:], rhs=nf[k][:, :NT],
                            start=(k == 0),
                            stop=(not has_bias and k == nk - 1))
                    if has_bias:
                        nc.tensor.matmul(
                            out=hp[:, :NT], lhsT=b1r[h][:], rhs=ones[:, :NT],
                            start=False, stop=True)
                    ht = mlp.tile([P, 512], F16, tag=f"h{h}", name=f"h_{n0}_{h}")
                    if h == 0:
                        nc.scalar.activation(
                            out=ht[:, :NT], in_=hp[:, :NT],
                            func=mybir.ActivationFunctionType.Relu,
                            bias=0.0, scale=1.0)
                    else:
                        nc.vector.tensor_scalar_max(
                            out=ht[:, :NT], in0=hp[:, :NT], scalar1=0.0)
                    hs.append(ht)
                xp = psum.tile([F, 512], F32, tag="xpsum", name=f"xp_{n0}")
                nk = D_H // P
                for k in range(nk):
                    nc.tensor.matmul(
                        out=xp[:, :NT], lhsT=w2[k][:], rhs=hs[k][:, :NT],
                        start=(k == 0),
                        stop=(not has_bias and k == nk - 1))
                if has_bias:
                    nc.tensor.matmul(
                        out=xp[:, :NT], lhsT=b2r[:], rhs=ones[:, :NT],
                        start=False, stop=True)
                xt = mlp.tile([F, 512], F32, tag="xt", name=f"xt_{n0}")
                nc.vector.tensor_scalar_mul(
                    out=xt[:, :NT], in0=xp[:, :NT], scalar1=c0)
                nc.sync.dma_start(
                    out=outT_d[:, n0:n0 + NT], in_=xt[:, :NT])
                n0 += NT

    nc.compile()
    return nc


_CACHE = {}


def kernel(node_feat, edge_index, W1, b1, W2, b2, temp):
    node_feat = np.asarray(node_feat, dtype=np.float32)
    edge_index = np.asarray(edge_index)
    W1 = np.ascontiguousarray(np.asarray(W1, dtype=np.float32))
    b1 = np.ascontiguousarray(np.asarray(b1, dtype=np.float32))
    W2 = np.ascontiguousarray(np.asarray(W2, dtype=np.float32))
    b2 = np.ascontiguousarray(np.asarray(b2, dtype=np.float32))
    temp = np.asarray(temp, dtype=np.float32)

    cj = _poly_coeffs(temp)
    degenerate = bool(np.max(np.abs(cj[1:])) <= 1e-9 * max(abs(cj[0]), 1.0))
    import os as _os
    if _os.environ.get("KFORCE_GENERAL", "") == "1":
        degenerate = False

    global LAST_RESULTS
    if degenerate:
        # p(t) == c0 identically: the aggregation contributes exactly
        # c_j * (...) = 0 for every j >= 1, so out = c0 * MLP(node_feat).
        nfT = np.zeros((NC, D_IN, NPC), dtype=np.float16)
        nf = node_feat.T.astype(np.float16)  # [D_IN, N]
        for c in range(NC):
            nfT[c, :, :NPC_REAL] = nf[:, c * NPC_REAL:(c + 1) * NPC_REAL]
        has_bias = bool(np.any(b1) or np.any(b2))
        key = ("mlp", float(cj[0]), has_bias)
        nc = _CACHE.get(key)
        if nc is None:
            nc = _build_mlp_nc(float(cj[0]), has_bias)
            _CACHE[key] = nc
        W1h = W1.astype(np.float16)
        b1h = b1.astype(np.float16)
        W2h = W2.astype(np.float16)
        b2h = b2.astype(np.float16)
        in_maps = []
        for c in range(NC):
            in_maps.append({
                "nfT": np.ascontiguousarray(nfT[c]),
                "W1": W1h, "b1": b1h, "W2": W2h, "b2": b2h,
            })
        res = bass_utils.run_bass_kernel_spmd(nc, in_maps,
                                              core_ids=list(range(NC)))
        LAST_RESULTS = res
        out = np.empty((N, F), dtype=np.float32)
        for c in range(NC):
            out[c * NPC_REAL:(c + 1) * NPC_REAL] = \
                np.asarray(res.results[c]["outT"])[:, :NPC_REAL].T
        return out

    prep = _host_prep(node_feat, edge_index, temp)

    key = (edge_index.tobytes()[:4096], temp.tobytes())
    nc = _CACHE.get(key)
    if nc is None:
        nc = _build_nc(prep["S_k"], prep["off"], prep["total_S"], prep["cj"])
        _CACHE[key] = nc

    in_maps = []
    for c in range(NC):
        in_maps.append({
            "nfT": np.ascontiguousarray(prep["nfT"][c]),
            "idx": np.ascontiguousarray(prep["idx_all"][c]),
            "degpk": np.ascontiguousarray(prep["degpk"][c]),
            "W1": W1, "b1": b1, "W2": W2, "b2": b2,
        })

    res = bass_utils.run_bass_kernel_spmd(nc, in_maps, core_ids=list(range(NC)))
    LAST_RESULTS = res
    out_cat = np.concatenate([r["out"] for r in res.results], axis=0)
    return np.ascontiguousarray(out_cat[prep["pos"]])


LAST_RESULTS = None
